# revision 17
# baseline (speedup 1.0000x reference)
"""Trainium2 Bass kernel for nn_CPSN (retrieval_knn PSM/PWG module).

Contract: kernel(**inputs) takes the FULL unsharded inputs (as produced by
setup_inputs) and returns the FULL output [2, b*q, s], distributing work
across 8 NeuronCores internally (data-parallel over the query dim q).

Algorithm per (q, s) pair (b=1, s=25, q=30, c=512, hw=361):
  O[x, y] = <f2n[:, x], f1n[:, y]>   (x = query pixel, y = support pixel)
  s21[x] = max_y O ; s12[y] = max_x O
  g1[x] = a1[argmax_y O[x, :]] ; g2[y] = a2[argmax_x O[:, y]]
  w = g1 * g2 ; out0 = mean(s12 * w) ; out1 = mean(s21 * w)

V2 design (fp16 data path):
- L2-normalization and the tiny meta-learner (a1/a2) run on HOST; the device
  receives fp16 normalized features.  fp16 matmuls run at 1 cycle/row on the
  PE (4x faster than fp32) with fp32 PSUM accumulation; host-validated
  end-to-end rel-err vs the fp32 reference is ~5.6e-3 (tolerance 2e-2).
- Both orientations (O[x,y] and its transpose) are computed by swapping
  matmul operands.  Per (pair, pixel-chunk): 4 c-chunk matmuls -> PSUM,
  ACT copies PSUM->SBUF fp16, DVE reduce_max gives the row max, and a fused
  scalar_tensor_tensor (is_ge -> one-hot, * attention row, accum) gives the
  argmax-gather without indices.  The stt pass is split across DVE (O phase)
  and GPSIMD (T phase) since InstTensorScalarPtr has no 2x DVE mode.
"""

import os
import sys

import numpy as np

for _p in ("/opt/trn_rl_repo", "/root/.axon_site/_ro/trn_rl_repo"):
    if os.path.isdir(_p) and _p not in sys.path:
        sys.path.insert(0, _p)

import concourse.bass as bass
import concourse.tile as tile
from concourse import bacc, library_config, mybir
from concourse.bass_utils import run_bass_kernel_spmd

# ---- problem constants (hardcoded per contract) ----
B, S, Q, C, H, W, TEMP = 1, 25, 30, 512, 19, 19, 64
HW = H * W  # 361
HWP = 362   # padded free pitch (4-byte aligned fp16 rows)
NCORES = 8
L = 4               # local (padded) query images per core; Q_PAD = 32
Q_PAD = NCORES * L
CCH = C // 128      # 4 contraction chunks
PCH = [(0, 128), (128, 128), (256, HW - 256)]  # pixel-dim partition chunks
GRP = 4             # pairs per PSUM tile (4 * 512 fp32 = 8KB = half of PSUM)
BN_EPS = 1e-5

F32 = mybir.dt.float32
F16 = mybir.dt.float16
AX_X = mybir.AxisListType.X
OP = mybir.AluOpType
AF = mybir.ActivationFunctionType


def build_program(variant="", repeat=1):
    """Build the (SPMD-shared) single-core bass program."""
    nc = bacc.Bacc(None, target_bir_lowering=False, debug=False)

    f1_d = nc.dram_tensor("f1", [S, C, HW], F16, kind="ExternalInput")
    f2_d = nc.dram_tensor("f2s", [L, C, HW], F16, kind="ExternalInput")
    # attention rows pre-broadcast across partitions on the host: plain
    # contiguous [128, HW] DMAs (per-tile broadcasts on gpsimd or via
    # stride-0 DMA measured 12-32us each on HW -- far too slow).
    a1r_d = nc.dram_tensor("a1r", [L, S, 128, HW], F16, kind="ExternalInput")
    a2r_d = nc.dram_tensor("a2r", [L, 128, HW], F16, kind="ExternalInput")
    out_d = nc.dram_tensor("out", [2 * L, S], F32, kind="ExternalOutput")

    with tile.TileContext(nc) as tc:
        from contextlib import ExitStack

        with ExitStack() as ctx:
            pp = ctx.enter_context(tc.tile_pool(name="pp", bufs=2, space="PSUM"))
            f1_pool = ctx.enter_context(tc.tile_pool(name="f1p", bufs=S))
            f2_pool = ctx.enter_context(tc.tile_pool(name="f2p", bufs=L))
            oc_pool = ctx.enter_context(tc.tile_pool(name="oc", bufs=4))
            row_pool = ctx.enter_context(tc.tile_pool(name="rows", bufs=8))
            a1bc_pool = ctx.enter_context(tc.tile_pool(name="a1bc", bufs=2 * S))
            a2bc_pool = ctx.enter_context(tc.tile_pool(name="a2bc", bufs=L))
            cols_pool = ctx.enter_context(tc.tile_pool(name="cols", bufs=1))
            cst_pool = ctx.enter_context(tc.tile_pool(name="cst", bufs=1))
            fin_pool = ctx.enter_context(tc.tile_pool(name="fin", bufs=6))

            mcol = cst_pool.tile([128, 1], F32, tag="mcol")
            nc.vector.memset(mcol[:], 1.0 / HW)

            # persistent fp16 feature tiles: [128, CCH, HW] per image.
            # f2 first: the T phase consumes f2t[*] plus f1t[ss] in ss order.
            f1t, f2t = {}, {}
            for l in range(L):
                t = f2_pool.tile([128, CCH, HW], F16, name=f"f2t{l}", tag="f2t")
                for c in range(CCH):
                    nc.sync.dma_start(t[:, c, :], f2_d[l, c * 128:(c + 1) * 128, :])
                f2t[l] = t
            for ss in range(S):
                t = f1_pool.tile([128, CCH, HW], F16, name=f"f1t{ss}", tag="f1t")
                for c in range(CCH):
                    nc.sync.dma_start(t[:, c, :], f1_d[ss, c * 128:(c + 1) * 128, :])
                f1t[ss] = t

            # a2 broadcast tiles (persist whole kernel)
            a2bc = []
            for l in range(L):
                t = a2bc_pool.tile([128, HWP], F16, name=f"a2bc{l}", tag="a2bc")
                nc.sync.dma_start(t[:, 0:HW], a2r_d[l])
                a2bc.append(t)

            # max / gather accumulator columns.
            # O phase (x on partitions): col = (l*3 + pch)*S + ss
            # T phase (y on partitions): col = (pch*S + ss)*L + l
            s21c = cols_pool.tile([128, 3 * S * L], F32, tag="s21c")
            g1c = cols_pool.tile([128, 3 * S * L], F32, tag="g1c")
            s12c = cols_pool.tile([128, 3 * S * L], F32, tag="s12c")
            g2c = cols_pool.tile([128, 3 * S * L], F32, tag="g2c")

            for _rep in range(repeat):
                # ---- T phase: T[y, x] per (ss, l); weights = f1 chunks ----
                for ss in range(S):
                    for pi, (y0, yp) in enumerate(PCH):
                        ps = pp.tile([128, L, 512], F32, name="psT", tag="ps")
                        if "nomm" not in variant:
                            for c in range(CCH):
                                for l in range(L):
                                    nc.tensor.matmul(
                                        ps[0:yp, l, 0:HW],
                                        f1t[ss][:, c, y0:y0 + yp],
                                        f2t[l][:, c, :],
                                        start=(c == 0), stop=(c == CCH - 1))
                        else:
                            nc.vector.memset(ps[:, :, :], 0.1)
                        if "nodve" in variant:
                            continue
                        if "nocp" in variant:
                            src = ps
                        else:
                            oc = oc_pool.tile([128, L, HWP], F16, name="ocT",
                                              tag="oc")
                            nc.scalar.activation(oc[0:yp, :, 0:HW],
                                                 ps[0:yp, :, 0:HW], AF.Copy)
                            src = oc
                        cb = (pi * S + ss) * L
                        if "normax" not in variant:
                            nc.vector.reduce_max(s12c[0:yp, cb:cb + L],
                                                 src[0:yp, :, 0:HW], axis=AX_X)
                        if "nostt" in variant:
                            continue
                        for l in range(L):
                            # TensorScalarPtr is not a legal Pool opcode;
                            # the gather pass must run on the Vector engine.
                            nc.vector.scalar_tensor_tensor(
                                src[0:yp, l, 0:HW], src[0:yp, l, 0:HW],
                                s12c[0:yp, cb + l:cb + l + 1],
                                a2bc[l][0:yp, 0:HW],
                                op0=OP.is_ge, op1=OP.mult,
                                accum_out=g2c[0:yp, cb + l:cb + l + 1])

                # ---- O phase: O[x, y] per (l, ss); weights = f2 chunks ----
                for l in range(L):
                    a1t = {}
                    if "nodve" not in variant:
                        for ss in range(S):
                            t = a1bc_pool.tile([128, HWP], F16,
                                               name=f"a1bc{l}_{ss}", tag="a1bc")
                            nc.sync.dma_start(t[:, 0:HW], a1r_d[l, ss])
                            a1t[ss] = t
                    for pi, (x0, xp) in enumerate(PCH):
                        for g0 in range(0, S, GRP):
                            grp = list(range(g0, min(g0 + GRP, S)))
                            ng = len(grp)
                            ps = pp.tile([128, L, 512], F32, name="psO", tag="ps")
                            if "nomm" not in variant:
                                for c in range(CCH):
                                    for j, ss in enumerate(grp):
                                        nc.tensor.matmul(
                                            ps[0:xp, j, 0:HW],
                                            f2t[l][:, c, x0:x0 + xp],
                                            f1t[ss][:, c, :],
                                            start=(c == 0), stop=(c == CCH - 1))
                            else:
                                nc.vector.memset(ps[:, :, :], 0.1)
                            if "nodve" in variant:
                                continue
                            if "nocp" in variant:
                                src = ps
                            else:
                                oc = oc_pool.tile([128, L, HWP], F16,
                                                  name="ocO", tag="oc")
                                nc.scalar.activation(oc[0:xp, 0:ng, 0:HW],
                                                     ps[0:xp, 0:ng, 0:HW],
                                                     AF.Copy)
                                src = oc
                            cb = (l * 3 + pi) * S + g0
                            if "normax" not in variant:
                                nc.vector.reduce_max(s21c[0:xp, cb:cb + ng],
                                                     src[0:xp, 0:ng, 0:HW],
                                                     axis=AX_X)
                            if "nostt" in variant:
                                continue
                            for j, ss in enumerate(grp):
                                nc.vector.scalar_tensor_tensor(
                                    src[0:xp, j, 0:HW], src[0:xp, j, 0:HW],
                                    s21c[0:xp, cb + j:cb + j + 1],
                                    a1t[ss][0:xp, 0:HW],
                                    op0=OP.is_ge, op1=OP.mult,
                                    accum_out=g1c[0:xp, cb + j:cb + j + 1])

                # ---- finals: w = g1*g2; out0 = mean(s12*w); out1 = mean(s21*w)
                if "nodve" in variant or "nostt" in variant or "normax" in variant:
                    continue
                for l in range(L):
                    fp1 = pp.tile([1, S], F32, name="fp1", tag="ps")
                    fp2 = pp.tile([1, S], F32, name="fp2", tag="ps")
                    for pi, (p0, pn) in enumerate(PCH):
                        ob = (l * 3 + pi) * S
                        g1 = g1c[0:pn, ob:ob + S]
                        s21 = s21c[0:pn, ob:ob + S]
                        tb = pi * S * L + l
                        s12b = s12c[0:pn, :]
                        g2b = g2c[0:pn, :]
                        s12 = bass.AP(s12b.tensor, s12b.offset + tb,
                                      [s12b.ap[0], [L, S]])
                        g2 = bass.AP(g2b.tensor, g2b.offset + tb,
                                     [g2b.ap[0], [L, S]])
                        wt = fin_pool.tile([128, S], F32, name="wt", tag="fin")
                        v1 = fin_pool.tile([128, S], F32, name="v1", tag="fin")
                        v2 = fin_pool.tile([128, S], F32, name="v2", tag="fin")
                        nc.vector.tensor_mul(wt[0:pn, :], g1, g2)
                        nc.vector.tensor_mul(v1[0:pn, :], s12, wt[0:pn, :])
                        nc.vector.tensor_mul(v2[0:pn, :], s21, wt[0:pn, :])
                        nc.tensor.matmul(fp1[:, :], mcol[0:pn, 0:1], v1[0:pn, :],
                                         start=(pi == 0), stop=(pi == 2))
                        nc.tensor.matmul(fp2[:, :], mcol[0:pn, 0:1], v2[0:pn, :],
                                         start=(pi == 0), stop=(pi == 2))
                    st1 = fin_pool.tile([1, S], F32, name=f"st1_{l}", tag="finst")
                    st2 = fin_pool.tile([1, S], F32, name=f"st2_{l}", tag="finst")
                    nc.scalar.activation(st1[:], fp1[0:1, :], AF.Copy)
                    nc.scalar.activation(st2[:], fp2[0:1, :], AF.Copy)
                    nc.sync.dma_start(out_d[l:l + 1, :], st1[0:1, :])
                    nc.sync.dma_start(out_d[L + l:L + l + 1, :], st2[0:1, :])

    nc.finalize()
    return nc


def _meta_learner_host(x, W1, g1, b1, m1, v1, W2, g2, b2, m2, v2):
    """x: [N, C, HW] -> [N, HW]  (two 1x1 convs + eval BN + ReLU on host)."""
    inv1 = g1 / np.sqrt(v1 + BN_EPS)
    bias1 = b1 - m1 * inv1
    y = np.einsum("tc,ncp->ntp", W1, x, dtype=np.float32)
    y = np.maximum(y * inv1[None, :, None] + bias1[None, :, None], 0.0)
    inv2 = g2 / np.sqrt(v2 + BN_EPS)
    bias2 = b2 - m2 * inv2
    z = np.einsum("ot,ntp->nop", W2, y, dtype=np.float32)
    z = np.maximum(z * inv2[None, :, None] + bias2[None, :, None], 0.0)
    return z[:, 0, :]


_NC_CACHE = [None]


def _prepare_in_maps(f1, f2, W1, g1, b1, m1, v1, W2, g2, b2, m2, v2):
    f1 = np.asarray(f1, np.float32).reshape(S, C, HW)
    f2 = np.asarray(f2, np.float32).reshape(Q, C, HW)
    W1 = np.asarray(W1, np.float32)
    W2 = np.asarray(W2, np.float32)
    g1, b1, m1, v1 = (np.asarray(a, np.float32) for a in (g1, b1, m1, v1))
    g2, b2, m2, v2 = (np.asarray(a, np.float32) for a in (g2, b2, m2, v2))

    # host: L2 normalization over the channel axis -> fp16
    def l2n(x):
        n = np.linalg.norm(x, axis=1, keepdims=True)
        return (x / np.maximum(n, 1e-12)).astype(np.float16)

    f1n = l2n(f1)
    f2n = l2n(f2)

    # host meta-learner (tiny): a1 [S, HW], a2 [Q, HW]
    a1 = _meta_learner_host(f1, W1, g1, b1, m1, v1, W2, g2, b2, m2, v2)
    a2 = _meta_learner_host(f2, W1, g1, b1, m1, v1, W2, g2, b2, m2, v2)

    f2p = np.zeros((Q_PAD, C, HW), np.float16)
    f2p[:Q] = f2n
    a2p = np.zeros((Q_PAD, HW), np.float32)
    a2p[:Q] = a2

    a1h = a1.astype(np.float16)
    in_maps = []
    for core in range(NCORES):
        qq = [core * L + l for l in range(L)]
        # host pre-broadcast across the 128 partitions (device-side
        # per-tile broadcasts are prohibitively slow)
        a1r = np.zeros((L, S, 128, HW), np.float16)
        a2r = np.zeros((L, 128, HW), np.float16)
        for l, q in enumerate(qq):
            if q < Q:
                for ss in range(S):
                    i1 = (q * S + ss) // Q  # faithful torch-layout quirk
                    a1r[l, ss] = a1h[i1][None, :]
                a2r[l] = a2p[q].astype(np.float16)[None, :]
        in_maps.append({
            "f1": f1n,
            "f2s": f2p[core * L:(core + 1) * L],
            "a1r": a1r,
            "a2r": a2r,
        })

    return in_maps


def _assemble(res):
    s1 = np.zeros((Q, S), np.float32)
    s2 = np.zeros((Q, S), np.float32)
    for core in range(NCORES):
        o = res.results[core]["out"].reshape(2, L, S)
        for l in range(L):
            q = core * L + l
            if q < Q:
                s1[q] = o[0, l]
                s2[q] = o[1, l]
    return np.stack([s1, s2])


def kernel(**inputs):
    in_maps = _prepare_in_maps(**inputs)
    if _NC_CACHE[0] is None:
        _NC_CACHE[0] = build_program()
    res = run_bass_kernel_spmd(_NC_CACHE[0], in_maps, list(range(NCORES)))
    return _assemble(res)


# revision 20
# speedup vs baseline: 1.1895x; 1.1895x over previous
"""Trainium2 Bass kernel for nn_CPSN (retrieval_knn PSM/PWG module).

Contract: kernel(**inputs) takes the FULL unsharded inputs (as produced by
setup_inputs) and returns the FULL output [2, b*q, s], distributing work
across 8 NeuronCores internally (data-parallel over the query dim q).

Algorithm per (q, s) pair (b=1, s=25, q=30, c=512, hw=361):
  O[x, y] = <f2n[:, x], f1n[:, y]>   (x = query pixel, y = support pixel)
  s21[x] = max_y O ; s12[y] = max_x O
  g1[x] = a1[argmax_y O[x, :]] ; g2[y] = a2[argmax_x O[:, y]]
  w = g1 * g2 ; out0 = mean(s12 * w) ; out1 = mean(s21 * w)

V2 design (fp16 data path):
- L2-normalization and the tiny meta-learner (a1/a2) run on HOST; the device
  receives fp16 normalized features.  fp16 matmuls run at 1 cycle/row on the
  PE (4x faster than fp32) with fp32 PSUM accumulation; host-validated
  end-to-end rel-err vs the fp32 reference is ~5.6e-3 (tolerance 2e-2).
- Both orientations (O[x,y] and its transpose) are computed by swapping
  matmul operands.  Per (pair, pixel-chunk): 4 c-chunk matmuls -> PSUM,
  ACT copies PSUM->SBUF fp16, DVE reduce_max gives the row max, and a fused
  scalar_tensor_tensor (is_ge -> one-hot, * attention row, accum) gives the
  argmax-gather without indices.  The stt pass is split across DVE (O phase)
  and GPSIMD (T phase) since InstTensorScalarPtr has no 2x DVE mode.
"""

import os
import sys

import numpy as np

for _p in ("/opt/trn_rl_repo", "/root/.axon_site/_ro/trn_rl_repo"):
    if os.path.isdir(_p) and _p not in sys.path:
        sys.path.insert(0, _p)

import concourse.bass as bass
import concourse.tile as tile
from concourse import bacc, library_config, mybir
from concourse.bass_utils import run_bass_kernel_spmd

# ---- problem constants (hardcoded per contract) ----
B, S, Q, C, H, W, TEMP = 1, 25, 30, 512, 19, 19, 64
HW = H * W  # 361
HWP = 362   # padded free pitch (4-byte aligned fp16 rows)
NCORES = 8
L = 4               # local (padded) query images per core; Q_PAD = 32
Q_PAD = NCORES * L
CCH = C // 128      # 4 contraction chunks
PCH = [(0, 128), (128, 128), (256, HW - 256)]  # pixel-dim partition chunks
GRP = 4             # pairs per PSUM tile (4 * 512 fp32 = 8KB = half of PSUM)
BN_EPS = 1e-5

F32 = mybir.dt.float32
F16 = mybir.dt.float16
AX_X = mybir.AxisListType.X
OP = mybir.AluOpType
AF = mybir.ActivationFunctionType


def build_program(variant="", repeat=1):
    """Build the (SPMD-shared) single-core bass program."""
    nc = bacc.Bacc(None, target_bir_lowering=False, debug=False)

    f1_d = nc.dram_tensor("f1", [S, C, HW], F16, kind="ExternalInput")
    f2_d = nc.dram_tensor("f2s", [L, C, HW], F16, kind="ExternalInput")
    # attention rows pre-broadcast across partitions on the host: plain
    # contiguous [128, HW] DMAs (per-tile broadcasts on gpsimd or via
    # stride-0 DMA measured 12-32us each on HW -- far too slow).
    a1r_d = nc.dram_tensor("a1r", [L, S, 128, HW], F16, kind="ExternalInput")
    a2r_d = nc.dram_tensor("a2r", [L, 128, HW], F16, kind="ExternalInput")
    out_d = nc.dram_tensor("out", [2 * L, S], F32, kind="ExternalOutput")

    with tile.TileContext(nc) as tc:
        from contextlib import ExitStack

        with ExitStack() as ctx:
            pp = ctx.enter_context(tc.tile_pool(name="pp", bufs=2, space="PSUM"))
            f1_pool = ctx.enter_context(tc.tile_pool(name="f1p", bufs=S))
            f2_pool = ctx.enter_context(tc.tile_pool(name="f2p", bufs=L))
            oc_pool = ctx.enter_context(tc.tile_pool(name="oc", bufs=4))
            row_pool = ctx.enter_context(tc.tile_pool(name="rows", bufs=8))
            a1bc_pool = ctx.enter_context(tc.tile_pool(name="a1bc", bufs=L * S))
            a2bc_pool = ctx.enter_context(tc.tile_pool(name="a2bc", bufs=L))
            cols_pool = ctx.enter_context(tc.tile_pool(name="cols", bufs=1))
            cst_pool = ctx.enter_context(tc.tile_pool(name="cst", bufs=1))
            fin_pool = ctx.enter_context(tc.tile_pool(name="fin", bufs=6))

            mcol = cst_pool.tile([128, 1], F32, tag="mcol")
            nc.vector.memset(mcol[:], 1.0 / HW)

            # persistent fp16 feature tiles: [128, CCH, HW] per image.
            # f2 first: the T phase consumes f2t[*] plus f1t[ss] in ss order.
            f1t, f2t = {}, {}
            for l in range(L):
                t = f2_pool.tile([128, CCH, HW], F16, name=f"f2t{l}", tag="f2t")
                for c in range(CCH):
                    nc.sync.dma_start(t[:, c, :], f2_d[l, c * 128:(c + 1) * 128, :])
                f2t[l] = t
            for ss in range(S):
                t = f1_pool.tile([128, CCH, HW], F16, name=f"f1t{ss}", tag="f1t")
                for c in range(CCH):
                    nc.sync.dma_start(t[:, c, :], f1_d[ss, c * 128:(c + 1) * 128, :])
                f1t[ss] = t

            # a2 broadcast tiles (persist whole kernel)
            a2bc = []
            for l in range(L):
                t = a2bc_pool.tile([128, HWP], F16, name=f"a2bc{l}", tag="a2bc")
                nc.sync.dma_start(t[:, 0:HW], a2r_d[l])
                a2bc.append(t)

            # a1 broadcast tiles: all (l, ss) loaded upfront so the O-phase
            # gather never waits on a DMA
            a1bc = {}
            if "nodve" not in variant:
                for l in range(L):
                    for ss in range(S):
                        t = a1bc_pool.tile([128, HWP], F16,
                                           name=f"a1bc{l}_{ss}", tag="a1bc")
                        nc.sync.dma_start(t[:, 0:HW], a1r_d[l, ss])
                        a1bc[(l, ss)] = t

            # max / gather accumulator columns.
            # O phase (x on partitions): col = (l*3 + pch)*S + ss
            # T phase (y on partitions): col = (pch*S + ss)*L + l
            s21c = cols_pool.tile([128, 3 * S * L], F32, tag="s21c")
            g1c = cols_pool.tile([128, 3 * S * L], F32, tag="g1c")
            s12c = cols_pool.tile([128, 3 * S * L], F32, tag="s12c")
            g2c = cols_pool.tile([128, 3 * S * L], F32, tag="g2c")

            for _rep in range(repeat):
                # ---- T phase: T[y, x] per (ss, l); weights = f1 chunks ----
                for ss in range(S):
                    for pi, (y0, yp) in enumerate(PCH):
                        ps = pp.tile([128, L, 512], F32, name="psT", tag="ps")
                        if "nomm" not in variant:
                            for c in range(CCH):
                                for l in range(L):
                                    nc.tensor.matmul(
                                        ps[0:yp, l, 0:HW],
                                        f1t[ss][:, c, y0:y0 + yp],
                                        f2t[l][:, c, :],
                                        start=(c == 0), stop=(c == CCH - 1))
                        else:
                            nc.vector.memset(ps[:, :, :], 0.1)
                        if "nodve" in variant:
                            continue
                        if "nocp" in variant:
                            src = ps
                        else:
                            oc = oc_pool.tile([128, L, HWP], F16, name="ocT",
                                              tag="oc")
                            nc.scalar.activation(oc[0:yp, :, 0:HW],
                                                 ps[0:yp, :, 0:HW], AF.Copy)
                            src = oc
                        cb = (pi * S + ss) * L
                        if "normax" not in variant:
                            nc.vector.reduce_max(s12c[0:yp, cb:cb + L],
                                                 src[0:yp, :, 0:HW], axis=AX_X)
                        if "nostt" in variant:
                            continue
                        for l in range(L):
                            # TensorScalarPtr is not a legal Pool opcode;
                            # the gather pass must run on the Vector engine.
                            nc.vector.scalar_tensor_tensor(
                                src[0:yp, l, 0:HW], src[0:yp, l, 0:HW],
                                s12c[0:yp, cb + l:cb + l + 1],
                                a2bc[l][0:yp, 0:HW],
                                op0=OP.is_ge, op1=OP.mult,
                                accum_out=g2c[0:yp, cb + l:cb + l + 1])

                # ---- O phase: O[x, y] per (l, ss); weights = f2 chunks ----
                for l in range(L):
                    if "noa1" in variant:
                        a1t = {ss: a2bc[l] for ss in range(S)}
                    else:
                        a1t = {ss: a1bc[(l, ss)] for ss in range(S)}
                    for pi, (x0, xp) in enumerate(PCH):
                        for g0 in range(0, S, GRP):
                            grp = list(range(g0, min(g0 + GRP, S)))
                            ng = len(grp)
                            ps = pp.tile([128, L, 512], F32, name="psO", tag="ps")
                            if "nomm" not in variant:
                                for c in range(CCH):
                                    for j, ss in enumerate(grp):
                                        nc.tensor.matmul(
                                            ps[0:xp, j, 0:HW],
                                            f2t[l][:, c, x0:x0 + xp],
                                            f1t[ss][:, c, :],
                                            start=(c == 0), stop=(c == CCH - 1))
                            else:
                                nc.vector.memset(ps[:, :, :], 0.1)
                            if "nodve" in variant:
                                continue
                            if "nocp" in variant:
                                src = ps
                            else:
                                oc = oc_pool.tile([128, L, HWP], F16,
                                                  name="ocO", tag="oc")
                                nc.scalar.activation(oc[0:xp, 0:ng, 0:HW],
                                                     ps[0:xp, 0:ng, 0:HW],
                                                     AF.Copy)
                                src = oc
                            cb = (l * 3 + pi) * S + g0
                            if "normax" not in variant:
                                nc.vector.reduce_max(s21c[0:xp, cb:cb + ng],
                                                     src[0:xp, 0:ng, 0:HW],
                                                     axis=AX_X)
                            if "nostt" in variant:
                                continue
                            for j, ss in enumerate(grp):
                                nc.vector.scalar_tensor_tensor(
                                    src[0:xp, j, 0:HW], src[0:xp, j, 0:HW],
                                    s21c[0:xp, cb + j:cb + j + 1],
                                    a1t[ss][0:xp, 0:HW],
                                    op0=OP.is_ge, op1=OP.mult,
                                    accum_out=g1c[0:xp, cb + j:cb + j + 1])

                # ---- finals: w = g1*g2; out0 = mean(s12*w); out1 = mean(s21*w)
                if "nodve" in variant or "nostt" in variant or "normax" in variant:
                    continue
                for l in range(L):
                    fp1 = pp.tile([1, S], F32, name="fp1", tag="ps")
                    fp2 = pp.tile([1, S], F32, name="fp2", tag="ps")
                    for pi, (p0, pn) in enumerate(PCH):
                        ob = (l * 3 + pi) * S
                        g1 = g1c[0:pn, ob:ob + S]
                        s21 = s21c[0:pn, ob:ob + S]
                        tb = pi * S * L + l
                        s12b = s12c[0:pn, :]
                        g2b = g2c[0:pn, :]
                        s12 = bass.AP(s12b.tensor, s12b.offset + tb,
                                      [s12b.ap[0], [L, S]])
                        g2 = bass.AP(g2b.tensor, g2b.offset + tb,
                                     [g2b.ap[0], [L, S]])
                        wt = fin_pool.tile([128, S], F32, name="wt", tag="fin")
                        v1 = fin_pool.tile([128, S], F32, name="v1", tag="fin")
                        v2 = fin_pool.tile([128, S], F32, name="v2", tag="fin")
                        nc.vector.tensor_mul(wt[0:pn, :], g1, g2)
                        nc.vector.tensor_mul(v1[0:pn, :], s12, wt[0:pn, :])
                        nc.vector.tensor_mul(v2[0:pn, :], s21, wt[0:pn, :])
                        nc.tensor.matmul(fp1[:, :], mcol[0:pn, 0:1], v1[0:pn, :],
                                         start=(pi == 0), stop=(pi == 2))
                        nc.tensor.matmul(fp2[:, :], mcol[0:pn, 0:1], v2[0:pn, :],
                                         start=(pi == 0), stop=(pi == 2))
                    st1 = fin_pool.tile([1, S], F32, name=f"st1_{l}", tag="finst")
                    st2 = fin_pool.tile([1, S], F32, name=f"st2_{l}", tag="finst")
                    nc.scalar.activation(st1[:], fp1[0:1, :], AF.Copy)
                    nc.scalar.activation(st2[:], fp2[0:1, :], AF.Copy)
                    nc.sync.dma_start(out_d[l:l + 1, :], st1[0:1, :])
                    nc.sync.dma_start(out_d[L + l:L + l + 1, :], st2[0:1, :])

    nc.finalize()
    return nc


def _meta_learner_host(x, W1, g1, b1, m1, v1, W2, g2, b2, m2, v2):
    """x: [N, C, HW] -> [N, HW]  (two 1x1 convs + eval BN + ReLU on host)."""
    inv1 = g1 / np.sqrt(v1 + BN_EPS)
    bias1 = b1 - m1 * inv1
    y = np.einsum("tc,ncp->ntp", W1, x, dtype=np.float32)
    y = np.maximum(y * inv1[None, :, None] + bias1[None, :, None], 0.0)
    inv2 = g2 / np.sqrt(v2 + BN_EPS)
    bias2 = b2 - m2 * inv2
    z = np.einsum("ot,ntp->nop", W2, y, dtype=np.float32)
    z = np.maximum(z * inv2[None, :, None] + bias2[None, :, None], 0.0)
    return z[:, 0, :]


_NC_CACHE = [None]


def _prepare_in_maps(f1, f2, W1, g1, b1, m1, v1, W2, g2, b2, m2, v2):
    f1 = np.asarray(f1, np.float32).reshape(S, C, HW)
    f2 = np.asarray(f2, np.float32).reshape(Q, C, HW)
    W1 = np.asarray(W1, np.float32)
    W2 = np.asarray(W2, np.float32)
    g1, b1, m1, v1 = (np.asarray(a, np.float32) for a in (g1, b1, m1, v1))
    g2, b2, m2, v2 = (np.asarray(a, np.float32) for a in (g2, b2, m2, v2))

    # host: L2 normalization over the channel axis -> fp16
    def l2n(x):
        n = np.linalg.norm(x, axis=1, keepdims=True)
        return (x / np.maximum(n, 1e-12)).astype(np.float16)

    f1n = l2n(f1)
    f2n = l2n(f2)

    # host meta-learner (tiny): a1 [S, HW], a2 [Q, HW]
    a1 = _meta_learner_host(f1, W1, g1, b1, m1, v1, W2, g2, b2, m2, v2)
    a2 = _meta_learner_host(f2, W1, g1, b1, m1, v1, W2, g2, b2, m2, v2)

    f2p = np.zeros((Q_PAD, C, HW), np.float16)
    f2p[:Q] = f2n
    a2p = np.zeros((Q_PAD, HW), np.float32)
    a2p[:Q] = a2

    a1h = a1.astype(np.float16)
    in_maps = []
    for core in range(NCORES):
        qq = [core * L + l for l in range(L)]
        # host pre-broadcast across the 128 partitions (device-side
        # per-tile broadcasts are prohibitively slow)
        a1r = np.zeros((L, S, 128, HW), np.float16)
        a2r = np.zeros((L, 128, HW), np.float16)
        for l, q in enumerate(qq):
            if q < Q:
                for ss in range(S):
                    i1 = (q * S + ss) // Q  # faithful torch-layout quirk
                    a1r[l, ss] = a1h[i1][None, :]
                a2r[l] = a2p[q].astype(np.float16)[None, :]
        in_maps.append({
            "f1": f1n,
            "f2s": f2p[core * L:(core + 1) * L],
            "a1r": a1r,
            "a2r": a2r,
        })

    return in_maps


def _assemble(res):
    s1 = np.zeros((Q, S), np.float32)
    s2 = np.zeros((Q, S), np.float32)
    for core in range(NCORES):
        o = res.results[core]["out"].reshape(2, L, S)
        for l in range(L):
            q = core * L + l
            if q < Q:
                s1[q] = o[0, l]
                s2[q] = o[1, l]
    return np.stack([s1, s2])


def kernel(**inputs):
    in_maps = _prepare_in_maps(**inputs)
    if _NC_CACHE[0] is None:
        _NC_CACHE[0] = build_program()
    res = run_bass_kernel_spmd(_NC_CACHE[0], in_maps, list(range(NCORES)))
    return _assemble(res)
